# revision 37
# baseline (speedup 1.0000x reference)
"""Multi-head self-attention (B=2, S=4096, D=512, H=8, Dk=64) on 8 TRN2 cores.

Sharding: data-parallel over batch x head-parallel. Core c handles batch
c//4 and head pair (2*(c%4), 2*(c%4)+1). Each core computes Q/K/V
projections for its 128 model dims, full attention for its two heads, and
a partial output projection against its 128 rows of Wo. The host sums the
four partial outputs per batch and adds bo.

The kernel is paced by the ACT engine (exp of 2*S^2 = 33.5M scores per
core at 1 elem/lane/cycle @ 1.2 GHz ~= 284us); everything else is
structured to keep ACT streaming back-to-back:
  - x arrives host-transposed (xT [512, S] bf16) and is DMA'd in S-chunks;
    Q/K/V projections run per-chunk so the first exp issues ~8us in
    (no xbar DMA transposes, no serial 48us prologue).
  - softmax denominator rides as a 65th column of V (ones), so the ctx
    matmul pair computes ctx+den with no separate den matmuls.
  - den rows are PE-transposed to [q-partition, head] form; reciprocal and
    the output normalization are per-partition DVE ops (no fp32r broadcast
    matmuls, no single-lane 512-wide reciprocals).
  - output projection runs per head (K=64 row-packed concurrent pair) on
    unnormalized ctx in bf16; DVE applies the two reciprocals and sums.
"""

import numpy as np
import ml_dtypes
from contextlib import ExitStack

import concourse.bass as bass
import concourse.tile as tile
from concourse import bacc, mybir
from concourse.bass_utils import run_bass_kernel_spmd

F32 = mybir.dt.float32
F16 = mybir.dt.float16
BF16 = mybir.dt.bfloat16
EXP = mybir.ActivationFunctionType.Exp
MULT = mybir.AluOpType.mult
ADD = mybir.AluOpType.add

V_VIA_TRANSPOSE = True

D_MODEL = 512
N_HEADS = 8
D_K = 64
N_CORES = 8
DL = 128          # local model dims per core (2 heads)
Q_BLK = 512       # query block (free dim of scores matmuls)
SCALE = 1.0 / np.sqrt(D_K).item()


def build_kernel(ctx, tc, S, use_mask, use_bq, use_bk, use_bv, d):
    nc = tc.nc
    SB = S // 128    # 128-wide s blocks
    QB = S // Q_BLK  # query blocks
    KB = SB          # key blocks of 128
    NCH = S // 512   # s-chunks for the load/projection pipeline

    sp = ctx.enter_context(tc.tile_pool(name="sp", bufs=1))
    psum = ctx.enter_context(tc.tile_pool(name="psum", bufs=1, space="PSUM"))
    # psum budget (8 banks): scores 2x[128,1024]=4, pc_h0/pc_h1 [65,512]=2,
    # tail/proj rotating tag = 2.

    # ---- constants ----
    # selector: den_row[0]=den_h0, den_row[32]=den_h1; tr = den_row_chunk^T
    # @ sel puts den_h0 in tr col 0 and den_h1 in tr col 1. fp16 keeps the
    # matmul single-pass (fp32 matmuls run LOW/HIGH double passes) at
    # ~5e-4 relative rounding on the denominator.
    sel = sp.tile([33, 2], F16, tag="sel")
    nc.vector.memset(sel, 0.0)
    nc.vector.memset(sel[0:1, 0:1], 1.0)
    nc.vector.memset(sel[32:33, 1:2], 1.0)

    # ---- PE warm-up: matmul activity lifts the HAM clock gate from 1.2
    # to 2.4 GHz while the first DMAs are in flight ----
    warm = sp.tile([128, 128], BF16, tag="warm")
    nc.vector.memset(warm, 0.0)
    for _ in range(14):
        pw = psum.tile([128, 128], F32, tag="tail", bufs=2, name="pw")
        nc.tensor.matmul(pw, warm, warm)

    # ---- DMAs in consumption order: x chunk 0 first (gates everything),
    # wo last (first needed ~45us in). x arrives chunk-major so each
    # chunk DMA is 128 descriptors of 4KB instead of 512 of 1KB. No DMA
    # transposes -> no xbar hazard. ----
    xt = sp.tile([128, NCH, 4, 512], BF16, tag="xt")

    def dma_chunk(c):
        nc.sync.dma_start(xt[:, c, :, :], d["xb"].ap()[c])

    dma_chunk(0)
    wq_sb = sp.tile([128, 4, 128], BF16, tag="wq")
    nc.sync.dma_start(wq_sb, d["wq"].ap().rearrange("(t p) d -> p t d", p=128))
    wk_sb = sp.tile([128, 4, 128], BF16, tag="wk")
    nc.sync.dma_start(wk_sb, d["wk"].ap().rearrange("(t p) d -> p t d", p=128))
    wv_sb = sp.tile([128, 4, 128], BF16, tag="wv")
    nc.sync.dma_start(wv_sb, d["wv"].ap().rearrange("(t p) d -> p t d", p=128))
    ident = sp.tile([128, 128], F32, tag="ident")
    nc.sync.dma_start(ident, d["ident"].ap())
    dma_chunk(1)
    if use_bq:
        bq_sb = sp.tile([128, 1], F32, tag="bq")
        nc.sync.dma_start(bq_sb, d["bq"].ap()[:, None])
    if use_bk:
        bk_sb = sp.tile([128, 1], F32, tag="bk")
        nc.sync.dma_start(bk_sb, d["bk"].ap()[:, None])
    if use_bv:
        bv_sb = sp.tile([128, 1], F32, tag="bv")
        nc.sync.dma_start(bv_sb, d["bv"].ap()[:, None])
    if use_mask:
        mb_sb = sp.tile([128, KB], F32, tag="mb")
        nc.sync.dma_start(mb_sb, d["mb"].ap())
    for c in range(2, NCH):
        dma_chunk(c)
    wo_sb = sp.tile([128, 512], BF16, tag="wo")
    nc.sync.dma_start(wo_sb, d["wo"].ap())

    qt = sp.tile([128, S], BF16, tag="qt")
    kt = sp.tile([128, S], BF16, tag="kt")
    # v_ext: per key block, cols 0:64 = V head0, 64 = ones, 65:129 = V
    # head1, 129 = ones -> the ctx matmul's 65th output row is the softmax
    # denominator.
    v_ext = sp.tile([128, SB, 130], BF16, tag="v")
    nc.vector.memset(v_ext[:, :, 64:65], 1.0)
    nc.vector.memset(v_ext[:, :, 129:130], 1.0)

    def project_qk(c, dst, w_sb, b_sb, ptag):
        cs = slice(c * 512, (c + 1) * 512)
        pp = psum.tile([128, 512], F32, tag=ptag, bufs=2, name="pp")
        for t in range(4):
            nc.tensor.matmul(pp, w_sb[:, t, :], xt[:, c, t, :],
                             start=(t == 0), stop=(t == 3))
        if b_sb is not None:
            nc.vector.tensor_scalar_add(dst[:, cs], pp, b_sb[:, 0:1])
        else:
            nc.vector.tensor_copy(dst[:, cs], pp)

    def project_v(c, ptag):
        # V computed [dk, s] like qt/kt (4 big matmuls, not 16 LDW-bound
        # small ones), then PE-transposed back to [s, dk] blocks.
        if V_VIA_TRANSPOSE:
            vt = psum.tile([128, 512], F32, tag=ptag, bufs=2, name="vt")
            for t in range(4):
                nc.tensor.matmul(vt, wv_sb[:, t, :], xt[:, c, t, :],
                                 start=(t == 0), stop=(t == 3))
            vtf = sp.tile([128, 512], F32, tag="vtf", bufs=2, name="vtf")
            if use_bv:
                nc.vector.tensor_scalar_add(vtf, vt, bv_sb[:, 0:1])
            else:
                nc.vector.tensor_copy(vtf, vt)
            tr_all = psum.tile([128, 4, 128], F32, tag=ptag, bufs=2,
                               name="tr_all")
            for j in range(4):
                nc.tensor.transpose(tr_all[:, j, :],
                                    vtf[:, j * 128:(j + 1) * 128], ident)
            nc.vector.tensor_copy(v_ext[:, 4 * c:4 * c + 4, 0:64],
                                  tr_all[:, :, 0:64])
            nc.vector.tensor_copy(v_ext[:, 4 * c:4 * c + 4, 65:129],
                                  tr_all[:, :, 64:128])
            return
        assert not use_bv
        for i in range(4):
            sb = 4 * c + i
            pv = psum.tile([128, 128], F32, tag=ptag, bufs=2, name="pv")
            for t in range(4):
                nc.tensor.matmul(
                    pv, xt[:, c, t, i * 128:(i + 1) * 128], wv_sb[:, t, :],
                    start=(t == 0), stop=(t == 3))
            nc.vector.tensor_copy(v_ext[:, sb, 0:64], pv[:, 0:64])
            nc.vector.tensor_copy(v_ext[:, sb, 65:129], pv[:, 64:128])

    def project_q(c, ptag="tail"):
        project_qk(c, qt, wq_sb, bq_sb if use_bq else None, ptag)

    def project_k(c, ptag="tail"):
        project_qk(c, kt, wk_sb, bk_sb if use_bk else None, ptag)

    # prologue: chunk 0 (+ qt chunk 1 for the warmup interleave).
    project_q(0)
    project_k(0)
    project_v(0, "tail")
    project_q(1)

    # ---- attention: flat (qb, kb) sequence ----
    # Warmup interleave: qb0 and qb1 alternate in 4-kb strides, so the
    # feasible exp work per projected x-chunk doubles (qb1 reuses the
    # same kt/v) and ACT streams while projections catch up. qb1's
    # accumulators borrow the "tail" psum buffers (no tail work exists
    # during warmup); projections during warmup borrow the "ps" buffers.
    seq = []
    for step in range(2 * (KB // 4)):
        q = step % 2
        k0 = (step // 2) * 4
        seq += [(q, k0 + j) for j in range(4)]
    for qb in range(2, QB):
        seq += [(qb, kb) for kb in range(KB)]
    assert len(seq) == QB * KB and len(set(seq)) == QB * KB

    # just-in-time projection emission: unit index -> closures
    emit = {}

    def emit_at(u, f):
        emit.setdefault(u, []).append(f)

    for c in range(1, NCH):  # kt/v chunk c first used at unit 8c (qb0
        emit_at(max(0, 8 * c - 6), lambda c=c: project_k(c, "ps"))
        emit_at(max(0, 8 * c - 5), lambda c=c: project_v(c, "ps"))
    if QB > 2:
        emit_at(52, lambda: project_q(2, "ps"))
    for qq in range(3, QB):  # needed at unit 64+(qq-2)*32
        emit_at(64 + (qq - 3) * 32 + 16, lambda qq=qq: project_q(qq))

    def scores_block(qb, kb):
        qs = slice(qb * Q_BLK, (qb + 1) * Q_BLK)
        ks = slice(kb * 128, (kb + 1) * 128)
        ps = psum.tile([128, 1024], F32, tag="ps", bufs=2, name="ps")
        nc.tensor.matmul(ps[:, 0:512], kt[0:64, ks], qt[0:64, qs])
        nc.tensor.matmul(ps[:, 512:1024], kt[64:128, ks], qt[64:128, qs])
        attn = sp.tile([128, 1024], BF16, tag="attn", bufs=4, name="attn")
        nc.scalar.activation(
            attn, ps, EXP, scale=SCALE,
            bias=mb_sb[:, kb:kb + 1] if use_mask else 0.0)
        return attn

    def extract_tail(qb, pc0, pc1):
        # PSUM extraction (DVE), emitted at the qb's last ctx matmul so
        # its WAR edges land before pc0/pc1 are reused. In the final
        # drain the Scalar engine is idle (no more exps), so the ctxn
        # extraction runs there, in parallel with the DVE den work.
        state = {}
        final = qb == QB - 1
        den_row = sp.tile([33, 512], F16, tag="den_row", bufs=2,
                          name="den_row")
        nc.vector.tensor_copy(den_row[0:1, :], pc0[64:65, :])
        nc.vector.tensor_copy(den_row[32:33, :], pc1[64:65, :])
        ctxn = sp.tile([128, 512], BF16, tag="ctxn", bufs=2, name="ctxn")
        if final:
            nc.scalar.copy(ctxn[0:64, :], pc0[0:64, :])
            nc.scalar.copy(ctxn[64:128, :], pc1[0:64, :])
        else:
            nc.vector.tensor_copy(ctxn[0:64, :], pc0[0:64, :])
            nc.vector.tensor_copy(ctxn[64:128, :], pc1[0:64, :])

        def stage_rcp(i):
            # den -> [q-partition, head] via a tiny PE matmul against the
            # selector; rcp then runs on all 128 lanes
            tr = psum.tile([128, 2], F32, tag="ps" if final else "tail",
                           bufs=2, name="tr")
            nc.tensor.matmul(tr, den_row[:, i * 128:(i + 1) * 128], sel)
            rcp = sp.tile([128, 2], F32, tag="rcp", bufs=4, name="rcp")
            nc.vector.reciprocal(rcp, tr)
            state[i] = rcp

        def stage_proj(i):
            rcp = state[i]
            cch = slice(i * 128, (i + 1) * 128)
            if final:
                # spread the drain's po allocations across four psum tag
                # pools (all free by now) so chunks pipeline instead of
                # serializing on 2-buffer WAR
                t0, b0 = ("ps", 2) if i % 2 == 0 else ("tail", 2)
                t1, b1 = ("pc0", 1) if i % 2 == 0 else ("pc1", 1)
            else:
                t0, b0 = t1, b1 = ("tail", 2)
            po0 = psum.tile([128, 512], F32, tag=t0, bufs=b0, name="po0")
            nc.tensor.matmul(po0, ctxn[0:64, cch], wo_sb[0:64, :])
            po1 = psum.tile([128, 512], F32, tag=t1, bufs=b1, name="po1")
            nc.tensor.matmul(po1, ctxn[64:128, cch], wo_sb[64:128, :])
            tmp = sp.tile([128, 512], F32, tag="tmp", bufs=2, name="tmp")
            if final:
                # Scalar engine is idle in the drain: run the first scale
                # there so it overlaps the DVE combine
                nc.scalar.mul(tmp, po1, rcp[:, 1:2])
            else:
                nc.vector.tensor_scalar_mul(tmp, po1, rcp[:, 1:2])
            ob = sp.tile([128, 512], F32, tag="ob", bufs=3, name="ob")
            nc.vector.scalar_tensor_tensor(
                ob, po0, rcp[:, 0:1], tmp, MULT, ADD)
            sb = qb * 4 + i
            nc.sync.dma_start(
                d["out"].ap()[sb * 128:(sb + 1) * 128, :], ob)

        stages = []
        for i in range(4):
            stages.append(lambda i=i: stage_rcp(i))
        for i in range(4):
            stages.append(lambda i=i: stage_proj(i))
        return stages

    # scores run 2 units ahead of ctx (matches the 2-deep ps rotation and
    # keeps ACT fed across qb boundaries)
    tails = []
    pcs = {}
    attn_q = [scores_block(*seq[0]), scores_block(*seq[1])]
    for i, (qb, kb) in enumerate(seq):
        if i + 2 < len(seq):
            attn_q.append(scores_block(*seq[i + 2]))
        for f in emit.get(i, []):
            f()
        if qb >= 2 and kb % 2 == 1 and tails:
            tails.pop(0)()
        if kb == 0:
            if qb == 1:
                pcs[qb] = (
                    psum.tile([65, 512], F32, tag="tail", bufs=2,
                              name="pc0b"),
                    psum.tile([65, 512], F32, tag="tail", bufs=2,
                              name="pc1b"))
            else:
                pcs[qb] = (
                    psum.tile([65, 512], F32, tag="pc0", bufs=1,
                              name="pc0"),
                    psum.tile([65, 512], F32, tag="pc1", bufs=1,
                              name="pc1"))
        pc0, pc1 = pcs[qb]
        attn = attn_q.pop(0)
        nc.tensor.matmul(pc0, v_ext[:, kb, 0:65], attn[:, 0:512],
                         start=(kb == 0), stop=(kb == KB - 1),
                         skip_group_check=True)
        nc.tensor.matmul(pc1, v_ext[:, kb, 65:130], attn[:, 512:1024],
                         start=(kb == 0), stop=(kb == KB - 1),
                         skip_group_check=True)
        if kb == KB - 1:
            tails.extend(extract_tail(qb, pc0, pc1))

    while tails:
        tails.pop(0)()


def build_program(S=4096, use_mask=False, use_bq=False, use_bk=False,
                  use_bv=False, enable_asserts=False):
    nc = bacc.Bacc("TRN2", target_bir_lowering=False, debug=False,
                   enable_asserts=enable_asserts, num_devices=N_CORES,
                   name="mha")
    d = {
        "xb": nc.dram_tensor("xb", [S // 512, 128, 4, 512], BF16,
                             kind="ExternalInput"),
        "wq": nc.dram_tensor("wq", [D_MODEL, DL], BF16, kind="ExternalInput"),
        "wk": nc.dram_tensor("wk", [D_MODEL, DL], BF16, kind="ExternalInput"),
        "wv": nc.dram_tensor("wv", [D_MODEL, DL], BF16, kind="ExternalInput"),
        "wo": nc.dram_tensor("wo", [DL, D_MODEL], BF16, kind="ExternalInput"),
        "ident": nc.dram_tensor("ident", [128, 128], F32,
                                kind="ExternalInput"),
        "out": nc.dram_tensor("out", [S, D_MODEL], F32, kind="ExternalOutput"),
    }
    if use_bq:
        d["bq"] = nc.dram_tensor("bq", [DL], F32, kind="ExternalInput")
    if use_bk:
        d["bk"] = nc.dram_tensor("bk", [DL], F32, kind="ExternalInput")
    if use_bv:
        d["bv"] = nc.dram_tensor("bv", [DL], F32, kind="ExternalInput")
    if use_mask:
        d["mb"] = nc.dram_tensor("mb", [128, S // 128], F32,
                                 kind="ExternalInput")
    with tile.TileContext(nc) as tc:
        with ExitStack() as ctx:
            build_kernel(ctx, tc, S, use_mask, use_bq, use_bk, use_bv, d)
    nc.compile()
    return nc


_cache = {}


def _program(key):
    if key not in _cache:
        _cache[key] = build_program(
            S=4096, use_mask=key[0], use_bq=key[1], use_bk=key[2],
            use_bv=key[3])
    return _cache[key]


def kernel(x, mask, Wq, bq, Wk, bk, Wv, bv, Wo, bo, _results_hook=None):
    x = np.asarray(x, np.float32)
    mask = np.asarray(mask)
    B, S, _ = x.shape
    use_mask = bool((mask == 0).any())
    use_bq = bool(np.asarray(bq).any())
    use_bk = bool(np.asarray(bk).any())
    use_bv = bool(np.asarray(bv).any())
    nc = _program((use_mask, use_bq, use_bk, use_bv))

    ident = np.eye(128, dtype=np.float32)
    # chunk-major x layout: [s-chunk, p, t, s] with d = t*128 + p, so each
    # per-chunk DMA moves 4KB-contiguous lines per partition
    xbs = {}
    for b in range(B):
        xbs[b] = np.ascontiguousarray(
            x[b].T.astype(ml_dtypes.bfloat16)
            .reshape(4, 128, S // 512, 512).transpose(2, 1, 0, 3))

    in_maps = []
    for c in range(N_CORES):
        b, j = divmod(c, N_CORES // B)
        ds = slice(j * DL, (j + 1) * DL)
        m = {
            "xb": xbs[b],
            "ident": ident,
            "wq": np.ascontiguousarray(Wq[:, ds]).astype(ml_dtypes.bfloat16),
            "wk": np.ascontiguousarray(Wk[:, ds]).astype(ml_dtypes.bfloat16),
            "wv": np.ascontiguousarray(Wv[:, ds]).astype(ml_dtypes.bfloat16),
            "wo": np.ascontiguousarray(Wo[ds, :]).astype(ml_dtypes.bfloat16),
        }
        if use_bq:
            m["bq"] = np.ascontiguousarray(bq[ds], dtype=np.float32)
        if use_bk:
            m["bk"] = np.ascontiguousarray(bk[ds], dtype=np.float32)
        if use_bv:
            m["bv"] = np.ascontiguousarray(bv[ds], dtype=np.float32)
        if use_mask:
            mb = np.where(np.asarray(mask[b]) == 0, -1e9, 0.0).astype(np.float32)
            m["mb"] = np.ascontiguousarray(mb.reshape(S // 128, 128).T)
        in_maps.append(m)

    res = run_bass_kernel_spmd(nc, in_maps, core_ids=list(range(N_CORES)))
    if _results_hook is not None:
        _results_hook(res)
    out = np.zeros((B, S, D_MODEL), np.float32)
    for c in range(N_CORES):
        b = c // (N_CORES // B)
        out[b] += res.results[c]["out"]
    out += np.asarray(bo, np.float32)
    return out


# revision 38
# speedup vs baseline: 1.0052x; 1.0052x over previous
"""Multi-head self-attention (B=2, S=4096, D=512, H=8, Dk=64) on 8 TRN2 cores.

Sharding: data-parallel over batch x head-parallel. Core c handles batch
c//4 and head pair (2*(c%4), 2*(c%4)+1). Each core computes Q/K/V
projections for its 128 model dims, full attention for its two heads, and
a partial output projection against its 128 rows of Wo. The host sums the
four partial outputs per batch and adds bo.

The kernel is paced by the ACT engine (exp of 2*S^2 = 33.5M scores per
core at 1 elem/lane/cycle @ 1.2 GHz ~= 284us); everything else is
structured to keep ACT streaming back-to-back:
  - x arrives host-transposed (xT [512, S] bf16) and is DMA'd in S-chunks;
    Q/K/V projections run per-chunk so the first exp issues ~8us in
    (no xbar DMA transposes, no serial 48us prologue).
  - softmax denominator rides as a 65th column of V (ones), so the ctx
    matmul pair computes ctx+den with no separate den matmuls.
  - den rows are PE-transposed to [q-partition, head] form; reciprocal and
    the output normalization are per-partition DVE ops (no fp32r broadcast
    matmuls, no single-lane 512-wide reciprocals).
  - output projection runs per head (K=64 row-packed concurrent pair) on
    unnormalized ctx in bf16; DVE applies the two reciprocals and sums.
"""

import numpy as np
import ml_dtypes
from contextlib import ExitStack

import concourse.bass as bass
import concourse.tile as tile
from concourse import bacc, mybir
from concourse.bass_utils import run_bass_kernel_spmd

F32 = mybir.dt.float32
F16 = mybir.dt.float16
BF16 = mybir.dt.bfloat16
EXP = mybir.ActivationFunctionType.Exp
MULT = mybir.AluOpType.mult
ADD = mybir.AluOpType.add

V_VIA_TRANSPOSE = False

D_MODEL = 512
N_HEADS = 8
D_K = 64
N_CORES = 8
DL = 128          # local model dims per core (2 heads)
Q_BLK = 512       # query block (free dim of scores matmuls)
SCALE = 1.0 / np.sqrt(D_K).item()


def build_kernel(ctx, tc, S, use_mask, use_bq, use_bk, use_bv, d):
    nc = tc.nc
    SB = S // 128    # 128-wide s blocks
    QB = S // Q_BLK  # query blocks
    KB = SB          # key blocks of 128
    NCH = S // 512   # s-chunks for the load/projection pipeline

    sp = ctx.enter_context(tc.tile_pool(name="sp", bufs=1))
    psum = ctx.enter_context(tc.tile_pool(name="psum", bufs=1, space="PSUM"))
    # psum budget (8 banks): scores 2x[128,1024]=4, pc_h0/pc_h1 [65,512]=2,
    # tail/proj rotating tag = 2.

    # ---- constants ----
    # selector: den_row[0]=den_h0, den_row[32]=den_h1; tr = den_row_chunk^T
    # @ sel puts den_h0 in tr col 0 and den_h1 in tr col 1. fp16 keeps the
    # matmul single-pass (fp32 matmuls run LOW/HIGH double passes) at
    # ~5e-4 relative rounding on the denominator.
    sel = sp.tile([33, 2], F16, tag="sel")
    nc.vector.memset(sel, 0.0)
    nc.vector.memset(sel[0:1, 0:1], 1.0)
    nc.vector.memset(sel[32:33, 1:2], 1.0)

    # ---- PE warm-up: matmul activity lifts the HAM clock gate from 1.2
    # to 2.4 GHz while the first DMAs are in flight ----
    warm = sp.tile([128, 128], BF16, tag="warm")
    nc.vector.memset(warm, 0.0)
    for _ in range(14):
        pw = psum.tile([128, 128], F32, tag="tail", bufs=2, name="pw")
        nc.tensor.matmul(pw, warm, warm)

    # ---- DMAs in consumption order: x chunk 0 first (gates everything),
    # wo last (first needed ~45us in). x arrives chunk-major so each
    # chunk DMA is 128 descriptors of 4KB instead of 512 of 1KB. No DMA
    # transposes -> no xbar hazard. ----
    xt = sp.tile([128, NCH, 4, 512], BF16, tag="xt")

    def dma_chunk(c):
        nc.sync.dma_start(xt[:, c, :, :], d["xb"].ap()[c])

    dma_chunk(0)
    wq_sb = sp.tile([128, 4, 128], BF16, tag="wq")
    nc.sync.dma_start(wq_sb, d["wq"].ap().rearrange("(t p) d -> p t d", p=128))
    wk_sb = sp.tile([128, 4, 128], BF16, tag="wk")
    nc.sync.dma_start(wk_sb, d["wk"].ap().rearrange("(t p) d -> p t d", p=128))
    wv_sb = sp.tile([128, 4, 128], BF16, tag="wv")
    nc.sync.dma_start(wv_sb, d["wv"].ap().rearrange("(t p) d -> p t d", p=128))
    ident = sp.tile([128, 128], F32, tag="ident")
    nc.sync.dma_start(ident, d["ident"].ap())
    dma_chunk(1)
    if use_bq:
        bq_sb = sp.tile([128, 1], F32, tag="bq")
        nc.sync.dma_start(bq_sb, d["bq"].ap()[:, None])
    if use_bk:
        bk_sb = sp.tile([128, 1], F32, tag="bk")
        nc.sync.dma_start(bk_sb, d["bk"].ap()[:, None])
    if use_bv:
        bv_sb = sp.tile([128, 1], F32, tag="bv")
        nc.sync.dma_start(bv_sb, d["bv"].ap()[:, None])
    if use_mask:
        mb_sb = sp.tile([128, KB], F32, tag="mb")
        nc.sync.dma_start(mb_sb, d["mb"].ap())
    for c in range(2, NCH):
        dma_chunk(c)
    wo_sb = sp.tile([128, 512], BF16, tag="wo")
    nc.sync.dma_start(wo_sb, d["wo"].ap())

    qt = sp.tile([128, S], BF16, tag="qt")
    kt = sp.tile([128, S], BF16, tag="kt")
    # v_ext: per key block, cols 0:64 = V head0, 64 = ones, 65:129 = V
    # head1, 129 = ones -> the ctx matmul's 65th output row is the softmax
    # denominator.
    v_ext = sp.tile([128, SB, 130], BF16, tag="v")
    nc.vector.memset(v_ext[:, :, 64:65], 1.0)
    nc.vector.memset(v_ext[:, :, 129:130], 1.0)

    def project_qk(c, dst, w_sb, b_sb, ptag):
        cs = slice(c * 512, (c + 1) * 512)
        pp = psum.tile([128, 512], F32, tag=ptag, bufs=2, name="pp")
        for t in range(4):
            nc.tensor.matmul(pp, w_sb[:, t, :], xt[:, c, t, :],
                             start=(t == 0), stop=(t == 3))
        if b_sb is not None:
            nc.vector.tensor_scalar_add(dst[:, cs], pp, b_sb[:, 0:1])
        else:
            nc.vector.tensor_copy(dst[:, cs], pp)

    def project_v(c, ptag):
        # V computed [dk, s] like qt/kt (4 big matmuls, not 16 LDW-bound
        # small ones), then PE-transposed back to [s, dk] blocks.
        if V_VIA_TRANSPOSE:
            vt = psum.tile([128, 512], F32, tag=ptag, bufs=2, name="vt")
            for t in range(4):
                nc.tensor.matmul(vt, wv_sb[:, t, :], xt[:, c, t, :],
                                 start=(t == 0), stop=(t == 3))
            vtf = sp.tile([128, 512], F32, tag="vtf", bufs=2, name="vtf")
            if use_bv:
                nc.vector.tensor_scalar_add(vtf, vt, bv_sb[:, 0:1])
            else:
                nc.vector.tensor_copy(vtf, vt)
            tr_all = psum.tile([128, 4, 128], F32, tag=ptag, bufs=2,
                               name="tr_all")
            for j in range(4):
                nc.tensor.transpose(tr_all[:, j, :],
                                    vtf[:, j * 128:(j + 1) * 128], ident)
            nc.vector.tensor_copy(v_ext[:, 4 * c:4 * c + 4, 0:64],
                                  tr_all[:, :, 0:64])
            nc.vector.tensor_copy(v_ext[:, 4 * c:4 * c + 4, 65:129],
                                  tr_all[:, :, 64:128])
            return
        assert not use_bv
        for i in range(4):
            sb = 4 * c + i
            pv = psum.tile([128, 128], F32, tag=ptag, bufs=2, name="pv")
            for t in range(4):
                nc.tensor.matmul(
                    pv, xt[:, c, t, i * 128:(i + 1) * 128], wv_sb[:, t, :],
                    start=(t == 0), stop=(t == 3))
            nc.vector.tensor_copy(v_ext[:, sb, 0:64], pv[:, 0:64])
            nc.vector.tensor_copy(v_ext[:, sb, 65:129], pv[:, 64:128])

    def project_q(c, ptag="tail"):
        project_qk(c, qt, wq_sb, bq_sb if use_bq else None, ptag)

    def project_k(c, ptag="tail"):
        project_qk(c, kt, wk_sb, bk_sb if use_bk else None, ptag)

    # prologue: chunk 0 (+ qt chunk 1 for the warmup interleave).
    project_q(0)
    project_k(0)
    project_v(0, "tail")
    project_q(1)

    # ---- attention: flat (qb, kb) sequence ----
    # Warmup interleave: qb0 and qb1 alternate in 4-kb strides, so the
    # feasible exp work per projected x-chunk doubles (qb1 reuses the
    # same kt/v) and ACT streams while projections catch up. qb1's
    # accumulators borrow the "tail" psum buffers (no tail work exists
    # during warmup); projections during warmup borrow the "ps" buffers.
    seq = []
    for step in range(2 * (KB // 4)):
        q = step % 2
        k0 = (step // 2) * 4
        seq += [(q, k0 + j) for j in range(4)]
    for qb in range(2, QB):
        seq += [(qb, kb) for kb in range(KB)]
    assert len(seq) == QB * KB and len(set(seq)) == QB * KB

    # just-in-time projection emission: unit index -> closures
    emit = {}

    def emit_at(u, f):
        emit.setdefault(u, []).append(f)

    for c in range(1, NCH):  # kt/v chunk c first used at unit 8c (qb0
        emit_at(max(0, 8 * c - 6), lambda c=c: project_k(c, "ps"))
        emit_at(max(0, 8 * c - 5), lambda c=c: project_v(c, "ps"))
    if QB > 2:
        emit_at(52, lambda: project_q(2, "ps"))
    for qq in range(3, QB):  # needed at unit 64+(qq-2)*32
        emit_at(64 + (qq - 3) * 32 + 16, lambda qq=qq: project_q(qq))

    def scores_block(qb, kb):
        qs = slice(qb * Q_BLK, (qb + 1) * Q_BLK)
        ks = slice(kb * 128, (kb + 1) * 128)
        ps = psum.tile([128, 1024], F32, tag="ps", bufs=2, name="ps")
        nc.tensor.matmul(ps[:, 0:512], kt[0:64, ks], qt[0:64, qs])
        nc.tensor.matmul(ps[:, 512:1024], kt[64:128, ks], qt[64:128, qs])
        attn = sp.tile([128, 1024], BF16, tag="attn", bufs=4, name="attn")
        nc.scalar.activation(
            attn, ps, EXP, scale=SCALE,
            bias=mb_sb[:, kb:kb + 1] if use_mask else 0.0)
        return attn

    def extract_tail(qb, pc0, pc1):
        # PSUM extraction (DVE), emitted at the qb's last ctx matmul so
        # its WAR edges land before pc0/pc1 are reused. In the final
        # drain the Scalar engine is idle (no more exps), so the ctxn
        # extraction runs there, in parallel with the DVE den work.
        state = {}
        final = qb == QB - 1
        den_row = sp.tile([33, 512], F16, tag="den_row", bufs=2,
                          name="den_row")
        nc.vector.tensor_copy(den_row[0:1, :], pc0[64:65, :])
        nc.vector.tensor_copy(den_row[32:33, :], pc1[64:65, :])
        ctxn = sp.tile([128, 512], BF16, tag="ctxn", bufs=2, name="ctxn")
        if final:
            nc.scalar.copy(ctxn[0:64, :], pc0[0:64, :])
            nc.scalar.copy(ctxn[64:128, :], pc1[0:64, :])
        else:
            nc.vector.tensor_copy(ctxn[0:64, :], pc0[0:64, :])
            nc.vector.tensor_copy(ctxn[64:128, :], pc1[0:64, :])

        def stage_rcp(i):
            # den -> [q-partition, head] via a tiny PE matmul against the
            # selector; rcp then runs on all 128 lanes
            tr = psum.tile([128, 2], F32, tag="ps" if final else "tail",
                           bufs=2, name="tr")
            nc.tensor.matmul(tr, den_row[:, i * 128:(i + 1) * 128], sel)
            rcp = sp.tile([128, 2], F32, tag="rcp", bufs=4, name="rcp")
            nc.vector.reciprocal(rcp, tr)
            state[i] = rcp

        def stage_proj(i):
            rcp = state[i]
            cch = slice(i * 128, (i + 1) * 128)
            if final:
                # spread the drain's po allocations across four psum tag
                # pools (all free by now) so chunks pipeline instead of
                # serializing on 2-buffer WAR
                t0, b0 = ("ps", 2) if i % 2 == 0 else ("tail", 2)
                t1, b1 = ("pc0", 1) if i % 2 == 0 else ("pc1", 1)
            else:
                t0, b0 = t1, b1 = ("tail", 2)
            po0 = psum.tile([128, 512], F32, tag=t0, bufs=b0, name="po0")
            nc.tensor.matmul(po0, ctxn[0:64, cch], wo_sb[0:64, :])
            po1 = psum.tile([128, 512], F32, tag=t1, bufs=b1, name="po1")
            nc.tensor.matmul(po1, ctxn[64:128, cch], wo_sb[64:128, :])
            tmp = sp.tile([128, 512], F32, tag="tmp", bufs=2, name="tmp")
            if final:
                # Scalar engine is idle in the drain: run the first scale
                # there so it overlaps the DVE combine
                nc.scalar.mul(tmp, po1, rcp[:, 1:2])
            else:
                nc.vector.tensor_scalar_mul(tmp, po1, rcp[:, 1:2])
            ob = sp.tile([128, 512], F32, tag="ob", bufs=3, name="ob")
            nc.vector.scalar_tensor_tensor(
                ob, po0, rcp[:, 0:1], tmp, MULT, ADD)
            sb = qb * 4 + i
            nc.sync.dma_start(
                d["out"].ap()[sb * 128:(sb + 1) * 128, :], ob)

        stages = []
        for i in range(4):
            stages.append(lambda i=i: stage_rcp(i))
        for i in range(4):
            stages.append(lambda i=i: stage_proj(i))
        return stages

    # scores run 2 units ahead of ctx (matches the 2-deep ps rotation and
    # keeps ACT fed across qb boundaries)
    tails = []
    pcs = {}
    attn_q = [scores_block(*seq[0]), scores_block(*seq[1])]
    for i, (qb, kb) in enumerate(seq):
        if i + 2 < len(seq):
            attn_q.append(scores_block(*seq[i + 2]))
        for f in emit.get(i, []):
            f()
        if qb >= 2 and kb % 2 == 1 and tails:
            tails.pop(0)()
        if kb == 0:
            if qb == 1:
                pcs[qb] = (
                    psum.tile([65, 512], F32, tag="tail", bufs=2,
                              name="pc0b"),
                    psum.tile([65, 512], F32, tag="tail", bufs=2,
                              name="pc1b"))
            else:
                pcs[qb] = (
                    psum.tile([65, 512], F32, tag="pc0", bufs=1,
                              name="pc0"),
                    psum.tile([65, 512], F32, tag="pc1", bufs=1,
                              name="pc1"))
        pc0, pc1 = pcs[qb]
        attn = attn_q.pop(0)
        nc.tensor.matmul(pc0, v_ext[:, kb, 0:65], attn[:, 0:512],
                         start=(kb == 0), stop=(kb == KB - 1),
                         skip_group_check=True)
        nc.tensor.matmul(pc1, v_ext[:, kb, 65:130], attn[:, 512:1024],
                         start=(kb == 0), stop=(kb == KB - 1),
                         skip_group_check=True)
        if kb == KB - 1:
            tails.extend(extract_tail(qb, pc0, pc1))

    while tails:
        tails.pop(0)()


def build_program(S=4096, use_mask=False, use_bq=False, use_bk=False,
                  use_bv=False, enable_asserts=False):
    nc = bacc.Bacc("TRN2", target_bir_lowering=False, debug=False,
                   enable_asserts=enable_asserts, num_devices=N_CORES,
                   name="mha")
    d = {
        "xb": nc.dram_tensor("xb", [S // 512, 128, 4, 512], BF16,
                             kind="ExternalInput"),
        "wq": nc.dram_tensor("wq", [D_MODEL, DL], BF16, kind="ExternalInput"),
        "wk": nc.dram_tensor("wk", [D_MODEL, DL], BF16, kind="ExternalInput"),
        "wv": nc.dram_tensor("wv", [D_MODEL, DL], BF16, kind="ExternalInput"),
        "wo": nc.dram_tensor("wo", [DL, D_MODEL], BF16, kind="ExternalInput"),
        "ident": nc.dram_tensor("ident", [128, 128], F32,
                                kind="ExternalInput"),
        "out": nc.dram_tensor("out", [S, D_MODEL], F32, kind="ExternalOutput"),
    }
    if use_bq:
        d["bq"] = nc.dram_tensor("bq", [DL], F32, kind="ExternalInput")
    if use_bk:
        d["bk"] = nc.dram_tensor("bk", [DL], F32, kind="ExternalInput")
    if use_bv:
        d["bv"] = nc.dram_tensor("bv", [DL], F32, kind="ExternalInput")
    if use_mask:
        d["mb"] = nc.dram_tensor("mb", [128, S // 128], F32,
                                 kind="ExternalInput")
    with tile.TileContext(nc) as tc:
        with ExitStack() as ctx:
            build_kernel(ctx, tc, S, use_mask, use_bq, use_bk, use_bv, d)
    nc.compile()
    return nc


_cache = {}


def _program(key):
    if key not in _cache:
        _cache[key] = build_program(
            S=4096, use_mask=key[0], use_bq=key[1], use_bk=key[2],
            use_bv=key[3])
    return _cache[key]


def kernel(x, mask, Wq, bq, Wk, bk, Wv, bv, Wo, bo, _results_hook=None):
    x = np.asarray(x, np.float32)
    mask = np.asarray(mask)
    B, S, _ = x.shape
    use_mask = bool((mask == 0).any())
    use_bq = bool(np.asarray(bq).any())
    use_bk = bool(np.asarray(bk).any())
    use_bv = bool(np.asarray(bv).any())
    nc = _program((use_mask, use_bq, use_bk, use_bv))

    ident = np.eye(128, dtype=np.float32)
    # chunk-major x layout: [s-chunk, p, t, s] with d = t*128 + p, so each
    # per-chunk DMA moves 4KB-contiguous lines per partition
    xbs = {}
    for b in range(B):
        xbs[b] = np.ascontiguousarray(
            x[b].T.astype(ml_dtypes.bfloat16)
            .reshape(4, 128, S // 512, 512).transpose(2, 1, 0, 3))

    in_maps = []
    for c in range(N_CORES):
        b, j = divmod(c, N_CORES // B)
        ds = slice(j * DL, (j + 1) * DL)
        m = {
            "xb": xbs[b],
            "ident": ident,
            "wq": np.ascontiguousarray(Wq[:, ds]).astype(ml_dtypes.bfloat16),
            "wk": np.ascontiguousarray(Wk[:, ds]).astype(ml_dtypes.bfloat16),
            "wv": np.ascontiguousarray(Wv[:, ds]).astype(ml_dtypes.bfloat16),
            "wo": np.ascontiguousarray(Wo[ds, :]).astype(ml_dtypes.bfloat16),
        }
        if use_bq:
            m["bq"] = np.ascontiguousarray(bq[ds], dtype=np.float32)
        if use_bk:
            m["bk"] = np.ascontiguousarray(bk[ds], dtype=np.float32)
        if use_bv:
            m["bv"] = np.ascontiguousarray(bv[ds], dtype=np.float32)
        if use_mask:
            mb = np.where(np.asarray(mask[b]) == 0, -1e9, 0.0).astype(np.float32)
            m["mb"] = np.ascontiguousarray(mb.reshape(S // 128, 128).T)
        in_maps.append(m)

    res = run_bass_kernel_spmd(nc, in_maps, core_ids=list(range(N_CORES)))
    if _results_hook is not None:
        _results_hook(res)
    out = np.zeros((B, S, D_MODEL), np.float32)
    for c in range(N_CORES):
        b = c // (N_CORES // B)
        out[b] += res.results[c]["out"]
    out += np.asarray(bo, np.float32)
    return out
